# revision 8
# baseline (speedup 1.0000x reference)
"""Cost-volume kernel for Trainium2 (Bass/Tile), SPMD over 8 NeuronCores.

out[b,c,d,h,w] = left[b,c,h,w] * right[b,c,h,w-d]  (0 where w < d), clipped.

Sharding: channels C=32 split 4-per-core (identical SPMD program, cores differ
only in input data). Each core computes its [4, 64, 160, 320] slab; the host
concatenates along C.

Per-core layout: rows (c,h) on the 128 SBUF partitions.
  - tiles 0..3: channel c, h in [0,128)          -> [128, W]
  - tile  4   : all 4 channels, h in [128,160)   -> [4*32, W] packed
The disparity shift is along W only, so rows are independent.

Compute: r is staged with a 64-column zero head (rpad[:, 0:64] = 0,
rpad[:, 64:384] = r), and the disparity axis is REVERSED (e = 63-d) so all
access-pattern strides stay positive:
    blk[p, e', w] = l[p, w] * rpad[p, 1 + e0 + e' + w]
                  = l[p, w] * r[p, w - d]          (0 where w < d)
One DVE tensor_tensor per (tile, 16-disparity group) computes the full-width
product INCLUDING the masked zeros (l times the zero head), so the output
needs no memsets and no ragged stores.

Stores: the per-core output is laid out h-major [C_LOC, H, E=64, W] with
e = 63-d. For one h row, a 16-disparity group is 16*320*4 = 20 KB contiguous
in DRAM, so each (tile, group) is ONE big DMA of fully-contiguous-per-row
streams. Measured on HW (K-repetition slope): contiguous stores run ~1.75x
faster than the same bytes as scattered ~1.2 KB diagonal runs, which real HBM
write bandwidth punishes (cost model does not).

The host gather reverses e and transposes (c, h, d, w) -> (c, d, h, w).
"""

import os

import numpy as np

os.environ.setdefault("NEURON_RT_RESET_CORES", "1")

import concourse.bass as bass
import concourse.tile as tile
from concourse import bacc, mybir
from concourse.bass_utils import run_bass_kernel_spmd

B, C, H, W = 1, 32, 160, 320
D = 64
N_CORES = 8
C_LOC = C // N_CORES          # 4 channels per core
H_MAIN = 128                  # h rows on partitions for the per-channel main tiles
H_TAIL = H - H_MAIN           # 32
N_TILES = C_LOC + 1           # 4 main + 1 packed tail
RPAD0 = D                     # zero-head columns of the staged right tiles
E_SPLIT = 4                   # disparity groups per tile (pipeline granularity)
EH = D // E_SPLIT             # 16 disparities per group

_cache = {}


def _build_program():
    nc = bacc.Bacc(
        "TRN2",
        target_bir_lowering=False,
        debug=False,
        enable_asserts=True,
        num_devices=N_CORES,
    )
    left_d = nc.dram_tensor(
        "left", [C_LOC, H, W], mybir.dt.float32, kind="ExternalInput"
    ).ap()
    right_d = nc.dram_tensor(
        "right", [C_LOC, H, W], mybir.dt.float32, kind="ExternalInput"
    ).ap()
    # h-major, e-reversed: out[c, h, e, w] = vol[c, d=63-e, h, w]
    out_d = nc.dram_tensor(
        "out", [C_LOC, H, D, W], mybir.dt.float32, kind="ExternalOutput"
    ).ap()

    lts = [
        nc.alloc_sbuf_tensor(f"lt{t}", [128, W], mybir.dt.float32).ap()
        for t in range(N_TILES)
    ]
    rts = [
        nc.alloc_sbuf_tensor(f"rt{t}", [128, RPAD0 + W], mybir.dt.float32).ap()
        for t in range(N_TILES)
    ]

    with tile.TileContext(nc) as tc:
        with tc.tile_pool(name="outp", bufs=8) as outp:
            # Zero heads: rpad[p, 64+k] = r[p, k], rpad[p, <64] = 0 implements
            # the w<d mask for free.
            for t in range(N_TILES):
                nc.vector.memset(rts[t][:, 0:RPAD0], 0.0)
            for t in range(C_LOC):
                nc.sync.dma_start(out=lts[t][:, :], in_=left_d[t, 0:H_MAIN, :])
                nc.sync.dma_start(out=rts[t][:, RPAD0:], in_=right_d[t, 0:H_MAIN, :])
            for c in range(C_LOC):
                p0 = c * H_TAIL
                nc.sync.dma_start(
                    out=lts[C_LOC][p0 : p0 + H_TAIL, :], in_=left_d[c, H_MAIN:H, :]
                )
                nc.sync.dma_start(
                    out=rts[C_LOC][p0 : p0 + H_TAIL, RPAD0:],
                    in_=right_d[c, H_MAIN:H, :],
                )

            for t in range(N_TILES):
                for s in range(E_SPLIT):
                    e0 = s * EH
                    blk = outp.tile(
                        [128, EH, W], mybir.dt.float32, name=f"blk_{t}_{s}", tag="blk"
                    )
                    bb = blk[:, :, :]
                    l_bc = lts[t][:, :].unsqueeze(1).broadcast_to([128, EH, W])
                    r_base = rts[t][:, :]
                    rpitch = r_base.ap[0][0]
                    r_win = bass.AP(
                        r_base.tensor,
                        r_base.offset + 1 + e0,
                        [[rpitch, 128], [1, EH], [1, W]],
                    )
                    nc.vector.tensor_mul(bb, l_bc, r_win)

                    if t < C_LOC:
                        nc.sync.dma_start(
                            out=out_d[t, 0:H_MAIN, e0 : e0 + EH, :], in_=bb
                        )
                    else:
                        for c in range(C_LOC):
                            bb_c = blk[c * H_TAIL : (c + 1) * H_TAIL, :, :]
                            nc.sync.dma_start(
                                out=out_d[c, H_MAIN:H, e0 : e0 + EH, :], in_=bb_c
                            )

    nc.compile()
    return nc


def kernel(**inputs):
    left = np.ascontiguousarray(np.asarray(inputs["left"], dtype=np.float32))
    right = np.ascontiguousarray(np.asarray(inputs["right"], dtype=np.float32))
    nd = int(np.asarray(inputs["num_disparities"]))
    assert left.shape == (B, C, H, W) and right.shape == (B, C, H, W)
    assert nd == D, f"kernel hardcodes num_disparities={D}, got {nd}"

    if "nc" not in _cache:
        _cache["nc"] = _build_program()
    nc = _cache["nc"]

    in_maps = [
        {
            "left": np.ascontiguousarray(left[0, i * C_LOC : (i + 1) * C_LOC]),
            "right": np.ascontiguousarray(right[0, i * C_LOC : (i + 1) * C_LOC]),
        }
        for i in range(N_CORES)
    ]
    res = run_bass_kernel_spmd(nc, in_maps, list(range(N_CORES)))
    _cache["last_results"] = res

    # per-core [C_LOC, H, E, W] (e = 63-d) -> (C, D, H, W)
    full = np.concatenate([np.asarray(r["out"]) for r in res.results], axis=0)
    full = np.ascontiguousarray(full[:, :, ::-1, :].transpose(0, 2, 1, 3))
    np.clip(full, -1000.0, 1000.0, out=full)
    return full[None]  # (1, 32, 64, 160, 320) float32
